# revision 19
# baseline (speedup 1.0000x reference)
"""2-layer GCN encoder on 8 Trainium2 NeuronCores (Bass/Tile).

Math: with dis = deg^{-1/2} (self-loops included), the GCN layer
    out = relu(D^{-1/2} A D^{-1/2} (X W) + b)
separates as
    out[v] = relu(dis[v] * (sum_{e: dst=v} dis[src]*X[src]) @ W + b)
so the per-edge norm disappears and both weight matmuls commute out of the
edge aggregation.  Aggregation is done as binary-selection matmuls on the
TensorEngine over dma_gather'ed rows of the dis-prescaled feature table.

Sharding: nodes are bin-packed by in-degree into 8 cores x 112 groups x 128
slots, with slot residues balanced so that every (group, src%4) edge cell
fits in 4 tiles of 128 (the %4 chunking is needed because dma_gather indices
are int16).  Both layers share one table order (tix) and hence one schedule:
layer 1 gathers from the dis-prescaled x table, layer 2 from the AllGathered
dis-prescaled relu(h1) table laid out in the same order.  The AllGather is
split into 4 quarter-pieces pipelined behind layer-1 compute.

Device-side notes:
  - the one-hot selection matrix S3 is built on DVE with the iota operand in
    PSUM so the op uses only dedicated SBUF ports and never locks GpSimd
    (SWDGE descriptor generation) out of the shared port pair;
  - bias is pre-loaded into PSUM as outer(1/dis, b) via a K=1 matmul, and the
    dst-side dis scaling + relu ride the Scalar engine's activation
    (dis*relu(x) == relu(dis*x));
  - gather idx tiles use a per-queue band layout (queue q's Q7 core pair only
    reads partitions [32q, 32q+32)) instead of 8x replication.
"""

import numpy as np
import ml_dtypes

import concourse.bacc as bacc
import concourse.tile as tile
import concourse.mybir as mybir
import concourse.bass as bass
from concourse.bass_utils import run_bass_kernel_spmd

# problem shapes (hardcoded per contract)
N = 100000
E = 1600000
IN_DIM, HID, OUT_DIM = 128, 128, 64

# schedule constants
P = 128           # partitions / tile edge count
NC_ = 8           # cores
G = 112           # groups per core
W = 7             # groups per batch
NB = 16           # batches per layer (W*NB == G)
TPC = 4           # tiles per (group, chunk)
NSEC = 4          # chunks (src table row mod 4)
SEC_T = W * TPC   # tiles per chunk section      = 28
BT = NSEC * SEC_T # tiles per batch              = 112
NODES_PC = G * P  # padded nodes per core        = 14336
TROWS = NC_ * NODES_PC  # shared table rows      = 114688
QTR = G // 4      # groups per AllGather quarter = 28
QROWS = QTR * P   # rows per quarter per core    = 3584
HCALLS = 2              # gather half-calls per chunk (descriptor-ring pipelining)
HT = SEC_T // HCALLS    # tiles per half-call = 14
IDXH = HT * P // 16     # wrapped idx cols per half-call = 112
IDXB = NSEC * HCALLS * IDXH  # idx cols per batch = 896
CELL_CAP = TPC * P      # max edges per (group, chunk) cell = 512

BF16 = ml_dtypes.bfloat16

_compiled = None  # cache across calls


# ----------------------------------------------------------------- host side

def _pack_nodes(deg):
    """Bin-pack nodes into 8*G bins (<=128 nodes each), balancing in-degree.

    Returns bin_of [N] (bin id), counts [NBINS].
    """
    import heapq
    NBINS = NC_ * G
    order = np.argsort(-deg, kind="stable")
    counts = np.zeros(NBINS, np.int64)
    loads = np.zeros(NBINS, np.float64)
    bin_of = np.empty(N, np.int64)
    h = [(0.0, b) for b in range(NBINS)]
    heapq.heapify(h)
    for n in order:
        while True:
            load, b = heapq.heappop(h)
            if counts[b] < P:
                break
        bin_of[n] = b
        counts[b] += 1
        loads[b] = load + deg[n]
        if counts[b] < P:
            heapq.heappush(h, (loads[b], b))
    # bins -> (core, gabs): snake-assign by load so core totals balance
    bins_sorted = np.argsort(-loads, kind="stable")
    core_of_bin = np.empty(NBINS, np.int64)
    gabs_of_bin = np.empty(NBINS, np.int64)
    next_g = np.zeros(NC_, np.int64)
    for r, b in enumerate(bins_sorted):
        rnd, pos = divmod(r, NC_)
        core = pos if rnd % 2 == 0 else NC_ - 1 - pos
        core_of_bin[b] = core
        gabs_of_bin[b] = next_g[core]
        next_g[core] += 1
    return bin_of, core_of_bin, gabs_of_bin


def _balance_residues(src_all, dstbin, bin_of, rng):
    """Assign each node a slot residue (slot % 4) so that every
    (dst bin, src residue) cell has <= CELL_CAP edges, respecting the
    <=32 nodes-per-residue-per-bin capacity.

    Returns res [N] in 0..3.
    """
    NBINS = NC_ * G
    # initial: random-ish balanced within each bin
    res = rng.permutation(N) % NSEC
    # enforce per-bin residue caps (<=32) by re-dealing within each bin
    order = np.argsort(bin_of, kind="stable")
    res_sorted = np.empty(N, np.int64)
    start = 0
    binned_counts = np.bincount(bin_of, minlength=NBINS)
    for b in range(NBINS):
        cnt = binned_counts[b]
        res_sorted[start:start + cnt] = np.arange(cnt) % NSEC
        start += cnt
    res[order] = res_sorted

    # per-src out-bin multiplicities (CSR over unique (src, dstbin))
    key = src_all * np.int64(NBINS) + dstbin
    ukey, mult = np.unique(key, return_counts=True)
    usrc = ukey // NBINS
    ubin = ukey % NBINS
    ptr = np.searchsorted(usrc, np.arange(N + 1))

    loads = np.bincount(dstbin * NSEC + res[src_all],
                        minlength=NBINS * NSEC).astype(np.int64)
    rescnt = np.zeros((NBINS, NSEC), np.int64)
    np.add.at(rescnt, (bin_of, res), 1)

    # edges grouped by (dstbin, residue-of-src) for mover lookup
    ecell = dstbin * NSEC + res[src_all]
    eorder = np.argsort(ecell, kind="stable")
    estart = np.searchsorted(ecell[eorder], np.arange(NBINS * NSEC + 1))

    for _pass in range(6):
        bad = np.flatnonzero(loads > CELL_CAP)
        if len(bad) == 0:
            break
        for cell in bad:
            excess = loads[cell] - CELL_CAP
            if excess <= 0:
                continue
            r = cell % NSEC
            movers = np.unique(src_all[eorder[estart[cell]:estart[cell + 1]]])
            for u in movers:
                if excess <= 0:
                    break
                if res[u] != r:
                    continue
                ub = ubin[ptr[u]:ptr[u + 1]]
                um = mult[ptr[u]:ptr[u + 1]]
                mybin = bin_of[u]
                best_r2, best_pen = -1, None
                for r2 in range(NSEC):
                    if r2 == r or rescnt[mybin, r2] >= P // NSEC:
                        continue
                    newloads = loads[ub * NSEC + r2] + um
                    pen = np.maximum(newloads - CELL_CAP, 0).sum()
                    if pen == 0 and (best_pen is None or best_pen > 0):
                        best_r2, best_pen = r2, 0
                        break
                    if best_pen is None or pen < best_pen:
                        best_r2, best_pen = r2, pen
                if best_r2 < 0 or (best_pen is not None and best_pen > 0):
                    continue
                loads[ub * NSEC + r] -= um
                loads[ub * NSEC + best_r2] += um
                rescnt[mybin, r] -= 1
                rescnt[mybin, best_r2] += 1
                res[u] = best_r2
                moved = um[ub == (cell // NSEC)].sum()
                excess -= moved
        # refresh mover lookup after each pass
        ecell = dstbin * NSEC + res[src_all]
        eorder = np.argsort(ecell, kind="stable")
        estart = np.searchsorted(ecell[eorder], np.arange(NBINS * NSEC + 1))

    return res, loads


def preprocess(x, edge_index):
    src = np.asarray(edge_index[0], dtype=np.int64)
    dst = np.asarray(edge_index[1], dtype=np.int64)
    loops = np.arange(N, dtype=np.int64)
    src_all = np.concatenate([src, loops])
    dst_all = np.concatenate([dst, loops])
    deg = np.bincount(dst_all, minlength=N).astype(np.float64)
    dis = (1.0 / np.sqrt(deg)).astype(np.float32)

    bin_of, core_of_bin, gabs_of_bin = _pack_nodes(deg)
    node_core = core_of_bin[bin_of]
    node_gabs = gabs_of_bin[bin_of]

    rng = np.random.default_rng(12345)
    dstbin = bin_of[dst_all]
    res, loads = _balance_residues(src_all, dstbin, bin_of, rng)
    assert loads.max() <= CELL_CAP, f"cell overflow: {loads.max()}"

    # assign slots within each bin: residue r nodes take slots r, r+4, ...
    binres = bin_of * NSEC + res
    order = np.argsort(binres, kind="stable")
    rank_in_binres = np.empty(N, np.int64)
    sorted_br = binres[order]
    starts = np.searchsorted(sorted_br, np.arange(NC_ * G * NSEC + 1))
    rnk = np.arange(N) - np.repeat(starts[:-1], np.diff(starts))
    rank_in_binres[order] = rnk
    node_slot = res + NSEC * rank_in_binres
    assert node_slot.max() < P

    # shared table order: core-major, matching the whole-tensor AllGather
    gid = node_core * NODES_PC + node_gabs * P + node_slot
    tix = gid

    # layer-1 table in tix order
    xs = np.zeros((TROWS, IN_DIM), BF16)
    xs[tix] = (np.asarray(x, np.float32) * dis[:, None]).astype(BF16)

    # shared edge schedule (same for both layers)
    ecore = node_core[dst_all]
    egabs = node_gabs[dst_all]
    eslot = node_slot[dst_all]
    src_tix = tix[src_all]
    chunk = src_tix % NSEC
    eidx = src_tix // NSEC
    cell = (ecore * G + egabs) * NSEC + chunk
    order = np.lexsort((eidx, cell))
    cell_s = cell[order]
    counts = np.bincount(cell, minlength=NC_ * G * NSEC)
    assert counts.max() <= CELL_CAP
    starts = np.concatenate([[0], np.cumsum(counts)[:-1]])
    rank = np.arange(len(cell_s)) - np.repeat(starts, counts)
    ch = cell_s % NSEC
    gg = (cell_s // NSEC) % G
    cr = cell_s // (NSEC * G)
    batch = gg // W
    gslot = gg % W
    tile_k = rank // P
    pos = rank % P
    T = batch * BT + ch * SEC_T + gslot * TPC + tile_k
    goff = cr * (NB * BT * P) + T * P + pos
    flat_idx = np.zeros(NC_ * NB * BT * P, np.int16)
    flat_dl = np.full(NC_ * NB * BT * P, P, np.int16)
    flat_idx[goff] = eidx[order].astype(np.int16)
    flat_dl[goff] = eslot[order].astype(np.int16)

    # wrapped idx layout: wrapped[p, s] = flat[s*16 + p%16], replicated x8.
    # Per batch: NSEC*HCALLS half-call slices of HT*P idxs each.
    fi = flat_idx.reshape(NC_, NB * NSEC * HCALLS, IDXH, 16)
    A = fi.transpose(0, 3, 1, 2).reshape(NC_, 16, NB * IDXB)
    idx_dram = np.tile(A, (1, 8, 1))  # [8, 128, NB*IDXB]
    dl_dram = (
        flat_dl.reshape(NC_, NB * BT, P).transpose(0, 2, 1).astype(np.float32)
    )  # [8, 128, NB*BT]

    dis_sb = np.zeros((NC_, P, G), np.float32)
    dis_sb[node_core, node_slot, node_gabs] = dis
    binv = np.zeros((NC_, G * P), np.float32)
    binv[node_core, node_gabs * P + node_slot] = 1.0 / dis

    return dict(
        xs=xs, idx=np.ascontiguousarray(idx_dram),
        dl=np.ascontiguousarray(dl_dram), dis_sb=dis_sb, binv=binv, gid=gid
    )


# --------------------------------------------------------------- device side

def build_program():
    f32 = mybir.dt.float32
    bf16 = mybir.dt.bfloat16
    i16 = mybir.dt.int16
    AO = mybir.AluOpType

    nc = bacc.Bacc(
        "TRN2", target_bir_lowering=False, debug=False, num_devices=NC_,
        num_swdge_queues=4, dynamic_dma_scratch_size=32768,
    )
    xs_d = nc.dram_tensor("xs", [TROWS, IN_DIM], bf16, kind="ExternalInput")
    idx_d = nc.dram_tensor("idx", [P, NB * IDXB], i16, kind="ExternalInput")
    dl_d = nc.dram_tensor("dl", [P, NB * BT], f32, kind="ExternalInput")
    dis_d = nc.dram_tensor("dis", [P, G], f32, kind="ExternalInput")
    dis2_d = nc.dram_tensor("dis2", [P, G], f32, kind="ExternalInput")
    binv_d = nc.dram_tensor("binv", [1, G * P], bf16, kind="ExternalInput")
    w1_d = nc.dram_tensor("w1", [IN_DIM, HID], bf16, kind="ExternalInput")
    w2_d = nc.dram_tensor("w2", [HID, OUT_DIM], bf16, kind="ExternalInput")
    b1_d = nc.dram_tensor("b1w", [1, HID], bf16, kind="ExternalInput")
    b2_d = nc.dram_tensor("b2w", [1, OUT_DIM], bf16, kind="ExternalInput")
    iota_d = nc.dram_tensor("iota", [P, P], f32, kind="ExternalInput")
    out_d = nc.dram_tensor("out", [NODES_PC, OUT_DIM], f32, kind="ExternalOutput")

    with tile.TileContext(nc) as tc:
        with tc.tile_pool(name="const", bufs=1) as cpool, \
             tc.tile_pool(name="io", bufs=3) as iopool, \
             tc.tile_pool(name="msgp", bufs=3) as mpool, \
             tc.tile_pool(name="sp", bufs=2) as spool, \
             tc.tile_pool(name="epi", bufs=3) as epool, \
             tc.tile_pool(name="psag", bufs=3, space="PSUM") as psag, \
             tc.tile_pool(name="psep", bufs=2, space="PSUM") as psep, \
             tc.tile_pool(name="psio", bufs=1, space="PSUM") as psio, \
             tc.tile_pool(name="dram", bufs=1, space="DRAM") as dpool:

            w1s = cpool.tile([IN_DIM, HID], bf16)
            nc.sync.dma_start(out=w1s[:], in_=w1_d[:])
            w2s = cpool.tile([HID, OUT_DIM], bf16)
            nc.sync.dma_start(out=w2s[:], in_=w2_d[:])
            b1s = cpool.tile([1, HID], bf16)
            nc.sync.dma_start(out=b1s[:], in_=b1_d[:])
            b2s = cpool.tile([1, OUT_DIM], bf16)
            nc.sync.dma_start(out=b2s[:], in_=b2_d[:])
            binv_s = cpool.tile([1, G * P], bf16)
            nc.sync.dma_start(out=binv_s[:], in_=binv_d[:])
            dis_s = cpool.tile([P, G], f32)
            nc.sync.dma_start(out=dis_s[:], in_=dis_d[:])
            dis2_s = cpool.tile([P, G], f32)
            nc.sync.dma_start(out=dis2_s[:], in_=dis2_d[:])
            iota_s = cpool.tile([P, P], f32)
            nc.sync.dma_start(out=iota_s[:], in_=iota_d[:])
            iota_ps = psio.tile([P, P], f32)
            nc.scalar.copy(out=iota_ps[:], in_=iota_s[:])

            gshard = dpool.tile([NODES_PC, HID], bf16)
            gf_p = [
                dpool.tile([NC_ * QROWS, HID], bf16, addr_space="Shared")
                for _ in range(4)
            ]
            gfull = dpool.tile([TROWS, HID], bf16)

            xs_v = xs_d[:].rearrange("(n f) d -> n f d", f=NSEC)
            gf_v = gfull.rearrange("(n f) d -> n f d", f=NSEC)
            gfull_cv = gfull.rearrange("(c q) d -> c q d", c=NC_)

            def layer(tbl_view, wsb, bsb, dout, sink, post_batch=None):
                for b in range(NB):
                    idx_t = iopool.tile([P, IDXB], i16, tag="idx")
                    nc.sync.dma_start(
                        out=idx_t[:], in_=idx_d[:, b * IDXB:(b + 1) * IDXB]
                    )
                    dl_t = iopool.tile([P, BT], f32, tag="dl")
                    nc.sync.dma_start(out=dl_t[:], in_=dl_d[:, b * BT:(b + 1) * BT])
                    msg = mpool.tile([P, BT, P], bf16, tag="msg")
                    for c in range(NSEC):
                        for h in range(HCALLS):
                            t0 = c * SEC_T + h * HT
                            s0 = (c * HCALLS + h) * IDXH
                            nc.gpsimd.dma_gather(
                                out_ap=msg[:, t0:t0 + HT, :],
                                in_ap=tbl_view[:, c, :],
                                idxs_ap=idx_t[:, s0:s0 + IDXH],
                                num_idxs=HT * P,
                                num_idxs_reg=HT * P,
                                elem_size=IN_DIM,
                                elem_step=IN_DIM * NSEC,
                                single_packet=False,
                                queue_num=c,
                            )
                    S3 = spool.tile([P, BT, P], bf16, tag="S3")
                    nc.vector.tensor_tensor(
                        out=S3[:],
                        in0=dl_t[:].unsqueeze(2).to_broadcast([P, BT, P]),
                        in1=iota_ps[:].unsqueeze(1).to_broadcast([P, BT, P]),
                        op=AO.is_equal,
                    )
                    for g in range(W):
                        ps = psag.tile([P, P], mybir.dt.float32, tag="agg")
                        for c in range(NSEC):
                            for k in range(TPC):
                                t = c * SEC_T + g * TPC + k
                                nc.tensor.matmul(
                                    out=ps[:],
                                    lhsT=msg[:, t, :],
                                    rhs=S3[:, t, :],
                                    start=(c == 0 and k == 0),
                                    stop=(c == NSEC - 1 and k == TPC - 1),
                                )
                        gabs = b * W + g
                        aggT = epool.tile([P, P], bf16, tag="aggT")
                        nc.scalar.copy(out=aggT[:], in_=ps[:])
                        po = psep.tile([P, dout], mybir.dt.float32, tag="po")
                        # bias pre-load: po = (b / dis)[dst, f] via outer product
                        nc.tensor.matmul(
                            out=po[:],
                            lhsT=binv_s[:, gabs * P:(gabs + 1) * P],
                            rhs=bsb[:],
                            start=True, stop=False,
                        )
                        nc.tensor.matmul(
                            out=po[:], lhsT=aggT[:], rhs=wsb[:], start=False, stop=True
                        )
                        sink(gabs, po)
                    if post_batch is not None:
                        post_batch(b)

            def sink1(gabs, po):
                # dis*relu(dis*agg + b1) == relu(dis2*(agg + b1/dis))
                gt = epool.tile([P, HID], mybir.dt.bfloat16, tag="gt")
                nc.scalar.activation(
                    out=gt[:], in_=po[:],
                    func=mybir.ActivationFunctionType.Relu,
                    scale=dis2_s[:, gabs:gabs + 1],
                )
                nc.sync.dma_start(
                    out=gshard[gabs * P:(gabs + 1) * P, :], in_=gt[:]
                )

            def sink2(gabs, po):
                # dis*agg + b2 == Copy(dis*(agg + b2/dis))
                o = epool.tile([P, OUT_DIM], mybir.dt.float32, tag="o")
                nc.scalar.activation(
                    out=o[:], in_=po[:],
                    func=mybir.ActivationFunctionType.Copy,
                    scale=dis_s[:, gabs:gabs + 1],
                )
                nc.sync.dma_start(
                    out=out_d[gabs * P:(gabs + 1) * P, :], in_=o[:]
                )

            def ag_piece(b):
                # fire AllGather quarter k once its 28 groups are sunk, then
                # scatter the piece into the core-major local table copy
                if (b + 1) % (NB // 4):
                    return
                k = b // (NB // 4)
                nc.gpsimd.collective_compute(
                    "AllGather",
                    mybir.AluOpType.bypass,
                    replica_groups=[list(range(NC_))],
                    ins=[gshard[k * QROWS:(k + 1) * QROWS, :].opt()],
                    outs=[gf_p[k].opt()],
                )
                src_v = gf_p[k].rearrange("(c q) d -> c q d", c=NC_)
                nc.sync.dma_start(
                    out=gfull_cv[:, k * QROWS:(k + 1) * QROWS, :],
                    in_=src_v[:],
                )

            layer(xs_v, w1s, b1s, HID, sink1, post_batch=ag_piece)
            layer(gf_v, w2s, b2s, OUT_DIM, sink2)

    nc.compile()
    return nc


# ------------------------------------------------------------------- runner

def run(inputs, trace=False):
    global _compiled
    x = np.asarray(inputs["x"], np.float32)
    edge_index = np.asarray(inputs["edge_index"])
    W1 = np.asarray(inputs["W1"], np.float32)
    b1 = np.asarray(inputs["b1"], np.float32)
    W2 = np.asarray(inputs["W2"], np.float32)
    b2 = np.asarray(inputs["b2"], np.float32)

    pp = preprocess(x, edge_index)

    if _compiled is None:
        _compiled = build_program()
    nc = _compiled

    iota = np.ascontiguousarray(
        np.broadcast_to(np.arange(P, dtype=np.float32), (P, P))
    )
    w1b = W1.astype(BF16)
    w2b = W2.astype(BF16)
    b1w = b1.reshape(1, HID).astype(BF16)
    b2w = b2.reshape(1, OUT_DIM).astype(BF16)

    in_maps = []
    for c in range(NC_):
        in_maps.append({
            "xs": pp["xs"],
            "idx": pp["idx"][c],
            "dl": pp["dl"][c],
            "dis": pp["dis_sb"][c],
            "dis2": pp["dis_sb"][c] ** 2,
            "binv": pp["binv"][c].reshape(1, G * P).astype(BF16),
            "w1": w1b,
            "w2": w2b,
            "b1w": b1w,
            "b2w": b2w,
            "iota": iota,
        })

    res = run_bass_kernel_spmd(
        nc, in_maps, core_ids=list(range(NC_)), trace=trace
    )
    allf = np.concatenate([res.results[c]["out"] for c in range(NC_)], axis=0)
    out = allf[pp["gid"]].astype(np.float32)
    return out, res


def kernel(**inputs):
    out, _ = run(inputs, trace=False)
    return out


# revision 21
# speedup vs baseline: 1.0710x; 1.0710x over previous
"""2-layer GCN encoder on 8 Trainium2 NeuronCores (Bass/Tile).

Math: with dis = deg^{-1/2} (self-loops included), the GCN layer
    out = relu(D^{-1/2} A D^{-1/2} (X W) + b)
separates as
    out[v] = relu(dis[v] * (sum_{e: dst=v} dis[src]*X[src]) @ W + b)
so the per-edge norm disappears and both weight matmuls commute out of the
edge aggregation.  Aggregation is done as binary-selection matmuls on the
TensorEngine over dma_gather'ed rows of the dis-prescaled feature table.

Sharding: nodes are bin-packed by in-degree into 8 cores x 112 groups x 128
slots, with slot residues balanced so that every (group, src%4) edge cell
fits in 4 tiles of 128 (the %4 chunking is needed because dma_gather indices
are int16).  Both layers share one table order (tix) and hence one schedule:
layer 1 gathers from the dis-prescaled x table, layer 2 from the AllGathered
dis-prescaled relu(h1) table laid out in the same order.  The AllGather is
split into 4 quarter-pieces pipelined behind layer-1 compute.

Device-side notes:
  - the one-hot selection matrix S3 is built on DVE with the iota operand in
    PSUM so the op uses only dedicated SBUF ports and never locks GpSimd
    (SWDGE descriptor generation) out of the shared port pair;
  - bias is pre-loaded into PSUM as outer(1/dis, b) via a K=1 matmul, and the
    dst-side dis scaling + relu ride the Scalar engine's activation
    (dis*relu(x) == relu(dis*x));
  - gather idx tiles use a per-queue band layout (queue q's Q7 core pair only
    reads partitions [32q, 32q+32)) instead of 8x replication.
"""

import numpy as np
import ml_dtypes

import concourse.bacc as bacc
import concourse.tile as tile
import concourse.mybir as mybir
import concourse.bass as bass
from concourse.bass_utils import run_bass_kernel_spmd

# problem shapes (hardcoded per contract)
N = 100000
E = 1600000
IN_DIM, HID, OUT_DIM = 128, 128, 64

# schedule constants
P = 128           # partitions / tile edge count
NC_ = 8           # cores
G = 112           # groups per core
W = 7             # groups per batch
NB = 16           # batches per layer (W*NB == G)
TPC = 4           # tiles per (group, chunk)
NSEC = 4          # chunks (src table row mod 4)
SEC_T = W * TPC   # tiles per chunk section      = 28
BT = NSEC * SEC_T # tiles per batch              = 112
NODES_PC = G * P  # padded nodes per core        = 14336
TROWS = NC_ * NODES_PC  # shared table rows      = 114688
QTR = G // 4      # groups per AllGather quarter = 28
QROWS = QTR * P   # rows per quarter per core    = 3584
HCALLS = 2              # gather half-calls per chunk (descriptor-ring pipelining)
HT = SEC_T // HCALLS    # tiles per half-call = 14
IDXH = HT * P // 16     # wrapped idx cols per half-call = 112
IDXB = NSEC * HCALLS * IDXH  # idx cols per batch = 896
CELL_CAP = TPC * P      # max edges per (group, chunk) cell = 512

BF16 = ml_dtypes.bfloat16

_compiled = None  # cache across calls


# ----------------------------------------------------------------- host side

def _pack_nodes(deg):
    """Bin-pack nodes into 8*G bins (<=128 nodes each), balancing in-degree.

    Returns bin_of [N] (bin id), counts [NBINS].
    """
    import heapq
    NBINS = NC_ * G
    order = np.argsort(-deg, kind="stable")
    counts = np.zeros(NBINS, np.int64)
    loads = np.zeros(NBINS, np.float64)
    bin_of = np.empty(N, np.int64)
    h = [(0.0, b) for b in range(NBINS)]
    heapq.heapify(h)
    for n in order:
        while True:
            load, b = heapq.heappop(h)
            if counts[b] < P:
                break
        bin_of[n] = b
        counts[b] += 1
        loads[b] = load + deg[n]
        if counts[b] < P:
            heapq.heappush(h, (loads[b], b))
    # bins -> (core, gabs): snake-assign by load so core totals balance
    bins_sorted = np.argsort(-loads, kind="stable")
    core_of_bin = np.empty(NBINS, np.int64)
    gabs_of_bin = np.empty(NBINS, np.int64)
    next_g = np.zeros(NC_, np.int64)
    for r, b in enumerate(bins_sorted):
        rnd, pos = divmod(r, NC_)
        core = pos if rnd % 2 == 0 else NC_ - 1 - pos
        core_of_bin[b] = core
        gabs_of_bin[b] = next_g[core]
        next_g[core] += 1
    return bin_of, core_of_bin, gabs_of_bin


def _balance_residues(src_all, dstbin, bin_of, rng):
    """Assign each node a slot residue (slot % 4) so that every
    (dst bin, src residue) cell has <= CELL_CAP edges, respecting the
    <=32 nodes-per-residue-per-bin capacity.

    Returns res [N] in 0..3.
    """
    NBINS = NC_ * G
    # initial: random-ish balanced within each bin
    res = rng.permutation(N) % NSEC
    # enforce per-bin residue caps (<=32) by re-dealing within each bin
    order = np.argsort(bin_of, kind="stable")
    res_sorted = np.empty(N, np.int64)
    start = 0
    binned_counts = np.bincount(bin_of, minlength=NBINS)
    for b in range(NBINS):
        cnt = binned_counts[b]
        res_sorted[start:start + cnt] = np.arange(cnt) % NSEC
        start += cnt
    res[order] = res_sorted

    # per-src out-bin multiplicities (CSR over unique (src, dstbin))
    key = src_all * np.int64(NBINS) + dstbin
    ukey, mult = np.unique(key, return_counts=True)
    usrc = ukey // NBINS
    ubin = ukey % NBINS
    ptr = np.searchsorted(usrc, np.arange(N + 1))

    loads = np.bincount(dstbin * NSEC + res[src_all],
                        minlength=NBINS * NSEC).astype(np.int64)
    rescnt = np.zeros((NBINS, NSEC), np.int64)
    np.add.at(rescnt, (bin_of, res), 1)

    # edges grouped by (dstbin, residue-of-src) for mover lookup
    ecell = dstbin * NSEC + res[src_all]
    eorder = np.argsort(ecell, kind="stable")
    estart = np.searchsorted(ecell[eorder], np.arange(NBINS * NSEC + 1))

    for _pass in range(6):
        bad = np.flatnonzero(loads > CELL_CAP)
        if len(bad) == 0:
            break
        for cell in bad:
            excess = loads[cell] - CELL_CAP
            if excess <= 0:
                continue
            r = cell % NSEC
            movers = np.unique(src_all[eorder[estart[cell]:estart[cell + 1]]])
            for u in movers:
                if excess <= 0:
                    break
                if res[u] != r:
                    continue
                ub = ubin[ptr[u]:ptr[u + 1]]
                um = mult[ptr[u]:ptr[u + 1]]
                mybin = bin_of[u]
                best_r2, best_pen = -1, None
                for r2 in range(NSEC):
                    if r2 == r or rescnt[mybin, r2] >= P // NSEC:
                        continue
                    newloads = loads[ub * NSEC + r2] + um
                    pen = np.maximum(newloads - CELL_CAP, 0).sum()
                    if pen == 0 and (best_pen is None or best_pen > 0):
                        best_r2, best_pen = r2, 0
                        break
                    if best_pen is None or pen < best_pen:
                        best_r2, best_pen = r2, pen
                if best_r2 < 0 or (best_pen is not None and best_pen > 0):
                    continue
                loads[ub * NSEC + r] -= um
                loads[ub * NSEC + best_r2] += um
                rescnt[mybin, r] -= 1
                rescnt[mybin, best_r2] += 1
                res[u] = best_r2
                moved = um[ub == (cell // NSEC)].sum()
                excess -= moved
        # refresh mover lookup after each pass
        ecell = dstbin * NSEC + res[src_all]
        eorder = np.argsort(ecell, kind="stable")
        estart = np.searchsorted(ecell[eorder], np.arange(NBINS * NSEC + 1))

    return res, loads


def preprocess(x, edge_index):
    src = np.asarray(edge_index[0], dtype=np.int64)
    dst = np.asarray(edge_index[1], dtype=np.int64)
    loops = np.arange(N, dtype=np.int64)
    src_all = np.concatenate([src, loops])
    dst_all = np.concatenate([dst, loops])
    deg = np.bincount(dst_all, minlength=N).astype(np.float64)
    dis = (1.0 / np.sqrt(deg)).astype(np.float32)

    bin_of, core_of_bin, gabs_of_bin = _pack_nodes(deg)
    node_core = core_of_bin[bin_of]
    node_gabs = gabs_of_bin[bin_of]

    rng = np.random.default_rng(12345)
    dstbin = bin_of[dst_all]
    res, loads = _balance_residues(src_all, dstbin, bin_of, rng)
    assert loads.max() <= CELL_CAP, f"cell overflow: {loads.max()}"

    # assign slots within each bin: residue r nodes take slots r, r+4, ...
    binres = bin_of * NSEC + res
    order = np.argsort(binres, kind="stable")
    rank_in_binres = np.empty(N, np.int64)
    sorted_br = binres[order]
    starts = np.searchsorted(sorted_br, np.arange(NC_ * G * NSEC + 1))
    rnk = np.arange(N) - np.repeat(starts[:-1], np.diff(starts))
    rank_in_binres[order] = rnk
    node_slot = res + NSEC * rank_in_binres
    assert node_slot.max() < P

    # shared table order: core-major, matching the whole-tensor AllGather
    gid = node_core * NODES_PC + node_gabs * P + node_slot
    tix = gid

    # layer-1 table in tix order
    xs = np.zeros((TROWS, IN_DIM), BF16)
    xs[tix] = (np.asarray(x, np.float32) * dis[:, None]).astype(BF16)

    # shared edge schedule (same for both layers)
    ecore = node_core[dst_all]
    egabs = node_gabs[dst_all]
    eslot = node_slot[dst_all]
    src_tix = tix[src_all]
    chunk = src_tix % NSEC
    eidx = src_tix // NSEC
    cell = (ecore * G + egabs) * NSEC + chunk
    order = np.lexsort((eidx, cell))
    cell_s = cell[order]
    counts = np.bincount(cell, minlength=NC_ * G * NSEC)
    assert counts.max() <= CELL_CAP
    starts = np.concatenate([[0], np.cumsum(counts)[:-1]])
    rank = np.arange(len(cell_s)) - np.repeat(starts, counts)
    ch = cell_s % NSEC
    gg = (cell_s // NSEC) % G
    cr = cell_s // (NSEC * G)
    batch = gg // W
    gslot = gg % W
    tile_k = rank // P
    pos = rank % P
    T = batch * BT + ch * SEC_T + gslot * TPC + tile_k
    goff = cr * (NB * BT * P) + T * P + pos
    flat_idx = np.zeros(NC_ * NB * BT * P, np.int16)
    flat_dl = np.full(NC_ * NB * BT * P, P, np.int16)
    flat_idx[goff] = eidx[order].astype(np.int16)
    flat_dl[goff] = eslot[order].astype(np.int16)

    # wrapped idx layout: wrapped[p, s] = flat[s*16 + p%16], replicated x8.
    # Per batch: NSEC*HCALLS half-call slices of HT*P idxs each.
    fi = flat_idx.reshape(NC_, NB * NSEC * HCALLS, IDXH, 16)
    A = fi.transpose(0, 3, 1, 2).reshape(NC_, 16, NB * IDXB)
    idx_dram = np.tile(A, (1, 8, 1))  # [8, 128, NB*IDXB]
    dl_dram = (
        flat_dl.reshape(NC_, NB * BT, P).transpose(0, 2, 1).astype(np.float32)
    )  # [8, 128, NB*BT]

    dis_sb = np.zeros((NC_, P, G), np.float32)
    dis_sb[node_core, node_slot, node_gabs] = dis
    binv = np.zeros((NC_, G * P), np.float32)
    binv[node_core, node_gabs * P + node_slot] = 1.0 / dis

    return dict(
        xs=xs, idx=np.ascontiguousarray(idx_dram),
        dl=np.ascontiguousarray(dl_dram), dis_sb=dis_sb, binv=binv, gid=gid
    )


# --------------------------------------------------------------- device side

def build_program():
    f32 = mybir.dt.float32
    bf16 = mybir.dt.bfloat16
    i16 = mybir.dt.int16
    AO = mybir.AluOpType

    nc = bacc.Bacc(
        "TRN2", target_bir_lowering=False, debug=False, num_devices=NC_,
        num_swdge_queues=4, dynamic_dma_scratch_size=32768,
    )
    xs_d = nc.dram_tensor("xs", [TROWS, IN_DIM], bf16, kind="ExternalInput")
    idx_d = nc.dram_tensor("idx", [P, NB * IDXB], i16, kind="ExternalInput")
    dl_d = nc.dram_tensor("dl", [P, NB * BT], f32, kind="ExternalInput")
    dis_d = nc.dram_tensor("dis", [P, G], f32, kind="ExternalInput")
    dis2_d = nc.dram_tensor("dis2", [P, G], f32, kind="ExternalInput")
    binv_d = nc.dram_tensor("binv", [1, G * P], bf16, kind="ExternalInput")
    w1_d = nc.dram_tensor("w1", [IN_DIM, HID], bf16, kind="ExternalInput")
    w2_d = nc.dram_tensor("w2", [HID, OUT_DIM], bf16, kind="ExternalInput")
    b1_d = nc.dram_tensor("b1w", [1, HID], bf16, kind="ExternalInput")
    b2_d = nc.dram_tensor("b2w", [1, OUT_DIM], bf16, kind="ExternalInput")
    iota_d = nc.dram_tensor("iota", [P, P], f32, kind="ExternalInput")
    out_d = nc.dram_tensor("out", [NODES_PC, OUT_DIM], f32, kind="ExternalOutput")

    with tile.TileContext(nc) as tc:
        with tc.tile_pool(name="const", bufs=1) as cpool, \
             tc.tile_pool(name="io", bufs=3) as iopool, \
             tc.tile_pool(name="msgp", bufs=3) as mpool, \
             tc.tile_pool(name="sp", bufs=2) as spool, \
             tc.tile_pool(name="epi", bufs=3) as epool, \
             tc.tile_pool(name="psag", bufs=3, space="PSUM") as psag, \
             tc.tile_pool(name="psep", bufs=2, space="PSUM") as psep, \
             tc.tile_pool(name="psio", bufs=1, space="PSUM") as psio, \
             tc.tile_pool(name="dram", bufs=1, space="DRAM") as dpool:

            w1s = cpool.tile([IN_DIM, HID], bf16)
            nc.sync.dma_start(out=w1s[:], in_=w1_d[:])
            w2s = cpool.tile([HID, OUT_DIM], bf16)
            nc.sync.dma_start(out=w2s[:], in_=w2_d[:])
            b1s = cpool.tile([1, HID], bf16)
            nc.sync.dma_start(out=b1s[:], in_=b1_d[:])
            b2s = cpool.tile([1, OUT_DIM], bf16)
            nc.sync.dma_start(out=b2s[:], in_=b2_d[:])
            binv_s = cpool.tile([1, G * P], bf16)
            nc.sync.dma_start(out=binv_s[:], in_=binv_d[:])
            dis_s = cpool.tile([P, G], f32)
            nc.sync.dma_start(out=dis_s[:], in_=dis_d[:])
            dis2_s = cpool.tile([P, G], f32)
            nc.sync.dma_start(out=dis2_s[:], in_=dis2_d[:])
            iota_s = cpool.tile([P, P], f32)
            nc.sync.dma_start(out=iota_s[:], in_=iota_d[:])
            iota_ps = psio.tile([P, P], f32)
            nc.scalar.copy(out=iota_ps[:], in_=iota_s[:])

            gshard = dpool.tile([NODES_PC, HID], bf16)
            gf_p = [
                dpool.tile([NC_ * QROWS, HID], bf16, addr_space="Shared",
                           name=f"gfp{k}")
                for k in range(4)
            ]
            gfull = dpool.tile([TROWS, HID], bf16)

            xs_v = xs_d[:].rearrange("(n f) d -> n f d", f=NSEC)
            gf_v = gfull.rearrange("(n f) d -> n f d", f=NSEC)
            gfull_cv = gfull.rearrange("(c q) d -> c q d", c=NC_)

            def layer(tbl_view, wsb, bsb, dout, sink, post_batch=None):
                for b in range(NB):
                    idx_t = iopool.tile([P, IDXB], i16, tag="idx")
                    nc.sync.dma_start(
                        out=idx_t[:], in_=idx_d[:, b * IDXB:(b + 1) * IDXB]
                    )
                    dl_t = iopool.tile([P, BT], f32, tag="dl")
                    nc.sync.dma_start(out=dl_t[:], in_=dl_d[:, b * BT:(b + 1) * BT])
                    msg = mpool.tile([P, BT, P], bf16, tag="msg")
                    for h in range(HCALLS):
                        for c in range(NSEC):
                            t0 = c * SEC_T + h * HT
                            s0 = (c * HCALLS + h) * IDXH
                            nc.gpsimd.dma_gather(
                                out_ap=msg[:, t0:t0 + HT, :],
                                in_ap=tbl_view[:, c, :],
                                idxs_ap=idx_t[:, s0:s0 + IDXH],
                                num_idxs=HT * P,
                                num_idxs_reg=HT * P,
                                elem_size=IN_DIM,
                                elem_step=IN_DIM * NSEC,
                                single_packet=False,
                                queue_num=c,
                            )
                    S3 = spool.tile([P, BT, P], bf16, tag="S3")
                    nc.vector.tensor_tensor(
                        out=S3[:],
                        in0=dl_t[:].unsqueeze(2).to_broadcast([P, BT, P]),
                        in1=iota_ps[:].unsqueeze(1).to_broadcast([P, BT, P]),
                        op=AO.is_equal,
                    )
                    for g in range(W):
                        ps = psag.tile([P, P], mybir.dt.float32, tag="agg")
                        for c in range(NSEC):
                            for k in range(TPC):
                                t = c * SEC_T + g * TPC + k
                                nc.tensor.matmul(
                                    out=ps[:],
                                    lhsT=msg[:, t, :],
                                    rhs=S3[:, t, :],
                                    start=(c == 0 and k == 0),
                                    stop=(c == NSEC - 1 and k == TPC - 1),
                                )
                        gabs = b * W + g
                        aggT = epool.tile([P, P], bf16, tag="aggT")
                        nc.scalar.copy(out=aggT[:], in_=ps[:])
                        po = psep.tile([P, dout], mybir.dt.float32, tag="po")
                        # bias pre-load: po = (b / dis)[dst, f] via outer product
                        nc.tensor.matmul(
                            out=po[:],
                            lhsT=binv_s[:, gabs * P:(gabs + 1) * P],
                            rhs=bsb[:],
                            start=True, stop=False,
                        )
                        nc.tensor.matmul(
                            out=po[:], lhsT=aggT[:], rhs=wsb[:], start=False, stop=True
                        )
                        sink(gabs, po)
                    if post_batch is not None:
                        post_batch(b)

            def sink1(gabs, po):
                # dis*relu(dis*agg + b1) == relu(dis2*(agg + b1/dis))
                gt = epool.tile([P, HID], mybir.dt.bfloat16, tag="gt")
                nc.scalar.activation(
                    out=gt[:], in_=po[:],
                    func=mybir.ActivationFunctionType.Relu,
                    scale=dis2_s[:, gabs:gabs + 1],
                )
                nc.sync.dma_start(
                    out=gshard[gabs * P:(gabs + 1) * P, :], in_=gt[:]
                )

            def sink2(gabs, po):
                # dis*agg + b2 == Copy(dis*(agg + b2/dis))
                o = epool.tile([P, OUT_DIM], mybir.dt.float32, tag="o")
                nc.scalar.activation(
                    out=o[:], in_=po[:],
                    func=mybir.ActivationFunctionType.Copy,
                    scale=dis_s[:, gabs:gabs + 1],
                )
                nc.sync.dma_start(
                    out=out_d[gabs * P:(gabs + 1) * P, :], in_=o[:]
                )

            def ag_piece(b):
                # fire AllGather quarter k once its 28 groups are sunk, then
                # scatter the piece into the core-major local table copy
                if (b + 1) % (NB // 4):
                    return
                k = b // (NB // 4)
                nc.gpsimd.collective_compute(
                    "AllGather",
                    mybir.AluOpType.bypass,
                    replica_groups=[list(range(NC_))],
                    ins=[gshard[k * QROWS:(k + 1) * QROWS, :].opt()],
                    outs=[gf_p[k].opt()],
                )
                src_v = gf_p[k].rearrange("(c q) d -> c q d", c=NC_)
                nc.sync.dma_start(
                    out=gfull_cv[:, k * QROWS:(k + 1) * QROWS, :],
                    in_=src_v[:],
                )

            layer(xs_v, w1s, b1s, HID, sink1, post_batch=ag_piece)
            layer(gf_v, w2s, b2s, OUT_DIM, sink2)

    nc.compile()
    return nc


# ------------------------------------------------------------------- runner

def run(inputs, trace=False):
    global _compiled
    x = np.asarray(inputs["x"], np.float32)
    edge_index = np.asarray(inputs["edge_index"])
    W1 = np.asarray(inputs["W1"], np.float32)
    b1 = np.asarray(inputs["b1"], np.float32)
    W2 = np.asarray(inputs["W2"], np.float32)
    b2 = np.asarray(inputs["b2"], np.float32)

    pp = preprocess(x, edge_index)

    if _compiled is None:
        _compiled = build_program()
    nc = _compiled

    iota = np.ascontiguousarray(
        np.broadcast_to(np.arange(P, dtype=np.float32), (P, P))
    )
    w1b = W1.astype(BF16)
    w2b = W2.astype(BF16)
    b1w = b1.reshape(1, HID).astype(BF16)
    b2w = b2.reshape(1, OUT_DIM).astype(BF16)

    in_maps = []
    for c in range(NC_):
        in_maps.append({
            "xs": pp["xs"],
            "idx": pp["idx"][c],
            "dl": pp["dl"][c],
            "dis": pp["dis_sb"][c],
            "dis2": pp["dis_sb"][c] ** 2,
            "binv": pp["binv"][c].reshape(1, G * P).astype(BF16),
            "w1": w1b,
            "w2": w2b,
            "b1w": b1w,
            "b2w": b2w,
            "iota": iota,
        })

    res = run_bass_kernel_spmd(
        nc, in_maps, core_ids=list(range(NC_)), trace=trace
    )
    allf = np.concatenate([res.results[c]["out"] for c in range(NC_)], axis=0)
    out = allf[pp["gid"]].astype(np.float32)
    return out, res


def kernel(**inputs):
    out, _ = run(inputs, trace=False)
    return out
